# revision 13
# baseline (speedup 1.0000x reference)
"""Trainium2 Bass kernel for the masked block-diagonal LSTM net.

Model structure (hardcoded from the problem spec):
  - x_seq [512, 64, 32], recurrent state HID=1088 = 34 blocks x 32.
  - U projections are masked so hidden block j only sees input feature j
    (block 0 additionally sees features 0,1 again via the interaction rows);
    hidden blocks 32,33 receive NO input projection at all.
  - V recurrent matrices are masked block-diagonal -> the 34 blocks evolve
    completely independently through the scan.

Sharding: hidden-block parallel. Cores 0..7 each own 4 input-driven blocks
(128 hidden rows) x the full batch 512. Layout on device is h^T:
[hid on partitions, batch on free dim], so the recurrent matmul, the gate
activations and the state updates all run at full 128-partition width with
N=512 columns and no transposes anywhere.

Blocks 32,33 are bias-only (no x dependence): their state is identical for
every batch element, so their scalar contribution to the readout (and the
tiny 16-feature static MLP + final sigmoid) is folded into the host-side
unshard step.

v2 restructure (vs the first working version):
  - t=0 recurrent matmuls skipped entirely (h0 == 0).
  - input-projection matmuls for step t+1 are emitted during step t and
    paired per gate across the two batch chunks so the PE can reuse the
    loaded weights and stay busy while the elementwise chain runs.
  - scalar_tensor_tensor ops (no DVE perf modes, ~594ns measured) replaced
    with tensor_scalar immediate ops (4x mode) and a real Tanh activation:
      cell gate: g2 = 2*sigmoid(2y) - 1 == tanh(y)   (weights pre-scaled x2)
      h update:  h  = o * tanh(c)                     (direct Tanh act)
"""

import sys

sys.path.insert(0, "/opt/trn_rl_repo")

import numpy as np

B = 512
T = 64
INPUT_SZ = 32
HPF = 32
INTER = [(0, 1), (2, 3)]
NB = INPUT_SZ + len(INTER)  # 34
HID = NB * HPF  # 1088
IN_SZ = INPUT_SZ + 2 * len(INTER)  # 36
F_STAT = 16
N_CORES = 8
BLOCKS_PER_CORE = 4
CORE_HID = BLOCKS_PER_CORE * HPF  # 128
CHUNKS = 2  # batch-column chunks per step (pipelining granularity)
CB = B // CHUNKS

_CACHE = {}


def _build_masks():
    um = np.zeros((IN_SZ, HID), np.float32)
    for i in range(INPUT_SZ):
        um[i, i * HPF : (i + 1) * HPF] = 1.0
    for i in range(0, len(INTER), 2):
        um[i + INPUT_SZ, i * HPF : (i + 1) * HPF] = 1.0
        um[i + INPUT_SZ + 1, i * HPF : (i + 1) * HPF] = 1.0
    vm = np.kron(np.eye(NB, dtype=np.float32), np.ones((HPF, HPF), np.float32))
    return um, vm


DEFAULT_CFG = dict(
    acts3=False,   # 3-way act split with gate banks (g,i,f,o); else fused
                   # 4-bank sigmoid with x2-prescaled cell gate
    chunks=2,      # batch-column chunks per step
    t1_eng="v",    # engine for f*c: "v" (vector) or "g" (gpsimd)
    inp_pos="after_acts",  # where inp(t+1) MMs are emitted:
                           # "after_acts" | "before_acts" | "after_chain"
    merged=False,  # emit chain(ch) right after act(ch) in one loop
    wbufs=4,
    sbufs=3,
)


def _build_program(repeat=1, loop_n=0, cfg=None):
    # repeat>1 duplicates the whole computation serially (same I/O).
    # loop_n>0 instead wraps ONE copy in a hardware For_i loop executing
    # loop_n times: program size stays constant, so wall-clock deltas
    # between two loop_n values isolate true device execution time from
    # the per-call NEFF dispatch overhead (which scales with program size).
    import concourse.bass as bass
    import concourse.tile as tile
    from concourse import bacc, mybir
    from contextlib import nullcontext

    f32 = mybir.dt.float32
    f16 = mybir.dt.float16
    ACT = mybir.ActivationFunctionType
    ALU = mybir.AluOpType

    cfg = dict(DEFAULT_CFG, **(cfg or {}))
    CHUNKS = cfg["chunks"]
    CB = B // CHUNKS

    nc = bacc.Bacc("TRN2", target_bir_lowering=False, debug=False)

    xf_d = nc.dram_tensor("xf", [5, T * B], f16, kind="ExternalInput").ap()
    wu_d = nc.dram_tensor("wu", [4, 5, CORE_HID], f16, kind="ExternalInput").ap()
    wv_d = nc.dram_tensor("wv", [4, CORE_HID, CORE_HID], f16, kind="ExternalInput").ap()
    oc_d = nc.dram_tensor("oc", [CORE_HID, 1], f16, kind="ExternalInput").ap()
    part_d = nc.dram_tensor("partial", [1, B], f32, kind="ExternalOutput").ap()

    with tile.TileContext(nc) as tc:
        with (
            tc.tile_pool(name="const", bufs=1) as cpool,
            tc.tile_pool(name="state", bufs=cfg["sbufs"]) as spool,
            tc.tile_pool(name="work", bufs=cfg["wbufs"]) as wpool,
            tc.tile_pool(name="psum", bufs=2, space="PSUM") as ppool,
        ):
            # DMA order: tiny input-projection weights first, then the
            # t=0 slice of xf so the first matmuls can issue while the
            # recurrent weights and the rest of xf stream in behind them.
            wu = []
            wv = []
            xf = cpool.tile([5, T * B], f16, tag="xf")
            for g in range(4):
                wut = cpool.tile([5, CORE_HID], f16, tag=f"wu{g}")
                nc.sync.dma_start(wut[:], wu_d[g])
                wu.append(wut)
            nc.sync.dma_start(xf[:, 0:B], xf_d[:, 0:B])
            for g in range(4):
                wvt = cpool.tile([CORE_HID, CORE_HID], f16, tag=f"wv{g}")
                nc.sync.dma_start(wvt[:], wv_d[g])
                wv.append(wvt)
            for lo, hi in ((1, 4), (4, 16), (16, 64)):
                nc.sync.dma_start(xf[:, lo * B : hi * B], xf_d[:, lo * B : hi * B])
            oc = cpool.tile([CORE_HID, 1], f16, tag="oc")
            nc.sync.dma_start(oc[:], oc_d[:])

            def emit_inp(ps_tiles, t, stop):
                # input projections for step t, paired per gate across
                # chunks so the PE can reuse the loaded weights
                for g in range(4):
                    for ch in range(CHUNKS):
                        nc.tensor.matmul(
                            ps_tiles[ch][:, g],
                            wu[g][:],
                            xf[:, t * B + ch * CB : t * B + (ch + 1) * CB],
                            start=True,
                            stop=stop,
                        )

            loop_cm = (lambda: tc.For_i(0, loop_n, 1)) if loop_n else None
            for rep in range(repeat):
              with loop_cm() if loop_cm else nullcontext():
                cs_t = [None] * CHUNKS
                hs_t = [None] * CHUNKS

                ps_cur = [
                    ppool.tile([128, 4, CB], f32, tag=f"ps{ch}", name=f"ps{ch}")
                    for ch in range(CHUNKS)
                ]
                emit_inp(ps_cur, 0, stop=True)

                def emit_rec(ch, t):
                    if t > 0:
                        for g in range(4):
                            nc.tensor.matmul(
                                ps_cur[ch][:, g],
                                wv[g][:],
                                hs_t[ch][:],
                                start=False,
                                stop=True,
                            )

                def emit_act(ch):
                    if cfg["acts3"]:
                        # banks are (g,i,f,o): direct tanh on the cell bank
                        # first (its matmul lands first), then sigmoid(i,f),
                        # then sigmoid(o) -- separate tiles so the c-path
                        # never waits on the o-gate activation
                        tg = wpool.tile([CORE_HID, CB], f16, tag=f"tg{ch}", name=f"tg{ch}")
                        nc.scalar.activation(tg[:], ps_cur[ch][:, 0], ACT.Tanh)
                        sif = wpool.tile([CORE_HID, 2, CB], f16, tag=f"sif{ch}", name=f"sif{ch}")
                        nc.scalar.activation(sif[:], ps_cur[ch][:, 1:3], ACT.Sigmoid)
                        so = wpool.tile([CORE_HID, CB], f16, tag=f"so{ch}", name=f"so{ch}")
                        nc.scalar.activation(so[:], ps_cur[ch][:, 3], ACT.Sigmoid)
                        return (tg, sif, so)
                    # one fused sigmoid over all 4 gate banks; the cell
                    # gate's weights are pre-scaled x2 so bank 3 yields
                    # g' = sigmoid(2y) with tanh(y) = 2g' - 1
                    ifog = wpool.tile([CORE_HID, 4, CB], f16, tag=f"ifog{ch}", name=f"ifog{ch}")
                    nc.scalar.activation(ifog[:], ps_cur[ch][:], ACT.Sigmoid)
                    return ifog

                def emit_chain(ch, ifog, first=False):
                    if cfg["acts3"]:
                        tg, sif, so = ifog
                        t1 = wpool.tile([CORE_HID, CB], f16, tag=f"t1{ch}", name=f"t1{ch}")
                        nc.vector.tensor_mul(t1[:], sif[:, 1], cs_t[ch][:])  # f*c
                        t2 = wpool.tile([CORE_HID, CB], f16, tag=f"t2{ch}", name=f"t2{ch}")
                        nc.vector.tensor_mul(t2[:], sif[:, 0], tg[:])  # i*tanh(y)
                        c_new = spool.tile([CORE_HID, CB], f16, tag=f"c{ch}", name=f"c{ch}")
                        nc.vector.tensor_add(c_new[:], t1[:], t2[:])
                        sc = wpool.tile([CORE_HID, CB], f16, tag=f"sc{ch}", name=f"sc{ch}")
                        nc.scalar.activation(sc[:], c_new[:], ACT.Tanh)
                        h_new = spool.tile([CORE_HID, CB], f16, tag=f"h{ch}", name=f"h{ch}")
                        nc.vector.tensor_mul(h_new[:], so[:], sc[:])
                        hs_t[ch] = h_new
                        cs_t[ch] = c_new
                        return
                    i_, f_, o_, g_ = (ifog[:, k] for k in range(4))
                    # tanh(y) = 2*sigmoid(2y) - 1 (4x-mode tensor_scalar)
                    g2 = wpool.tile([CORE_HID, CB], f16, tag=f"g2{ch}", name=f"g2{ch}")
                    nc.vector.tensor_scalar(
                        g2[:], g_, 2.0, -1.0, ALU.mult, ALU.add
                    )
                    c_new = spool.tile([CORE_HID, CB], f16, tag=f"c{ch}", name=f"c{ch}")
                    if first:
                        # c0 == 0: c1 = i * tanh(y) directly
                        nc.vector.tensor_mul(c_new[:], i_, g2[:])
                    else:
                        t1 = wpool.tile([CORE_HID, CB], f16, tag=f"t1{ch}", name=f"t1{ch}")
                        t1_eng = nc.vector if cfg["t1_eng"] == "v" else nc.gpsimd
                        t1_eng.tensor_mul(t1[:], f_, cs_t[ch][:])  # f*c
                        t2 = wpool.tile([CORE_HID, CB], f16, tag=f"t2{ch}", name=f"t2{ch}")
                        nc.vector.tensor_mul(t2[:], i_, g2[:])  # i*tanh(y)
                        nc.vector.tensor_add(c_new[:], t1[:], t2[:])
                    sc = wpool.tile([CORE_HID, CB], f16, tag=f"sc{ch}", name=f"sc{ch}")
                    nc.scalar.activation(sc[:], c_new[:], ACT.Tanh)
                    h_new = spool.tile([CORE_HID, CB], f16, tag=f"h{ch}", name=f"h{ch}")
                    nc.vector.tensor_mul(h_new[:], o_, sc[:])
                    hs_t[ch] = h_new
                    cs_t[ch] = c_new

                for t in range(T):
                    def emit_inp_next():
                        if t + 1 < T:
                            pn = [
                                ppool.tile([128, 4, CB], f32, tag=f"ps{ch}", name=f"ps{ch}")
                                for ch in range(CHUNKS)
                            ]
                            emit_inp(pn, t + 1, stop=False)
                            return pn
                        return None

                    ps_next = None
                    if cfg["inp_pos"] == "before_acts":
                        for ch in range(CHUNKS):
                            emit_rec(ch, t)
                        ps_next = emit_inp_next()
                        ifogs = [emit_act(ch) for ch in range(CHUNKS)]
                        for ch in range(CHUNKS):
                            emit_chain(ch, ifogs[ch], first=(t == 0))
                    elif cfg["merged"]:
                        ifogs = [None] * CHUNKS
                        for ch in range(CHUNKS):
                            emit_rec(ch, t)
                            ifogs[ch] = emit_act(ch)
                            if ch == 0 and cfg["inp_pos"] == "after_acts":
                                ps_next = emit_inp_next()
                            emit_chain(ch, ifogs[ch], first=(t == 0))
                        if ps_next is None:
                            ps_next = emit_inp_next()
                    else:
                        ifogs = []
                        for ch in range(CHUNKS):
                            emit_rec(ch, t)
                            ifogs.append(emit_act(ch))
                        if cfg["inp_pos"] == "after_acts":
                            ps_next = emit_inp_next()
                        for ch in range(CHUNKS):
                            emit_chain(ch, ifogs[ch], first=(t == 0))
                        if cfg["inp_pos"] == "after_chain":
                            ps_next = emit_inp_next()

                    ps_cur = ps_next

                # readout partial: oc^T @ h  -> [1, B]
                outsb = wpool.tile([1, B], f32, tag="outsb")
                for ch in range(CHUNKS):
                    pr = ppool.tile([128, 4, CB], f32, tag=f"ps{ch}")
                    nc.tensor.matmul(
                        pr[0:1, 0], oc[:], hs_t[ch][:], start=True, stop=True
                    )
                    nc.vector.tensor_copy(outsb[:, ch * CB : (ch + 1) * CB], pr[0:1, 0])
                nc.sync.dma_start(part_d[:], outsb[:])

    nc.compile()
    return nc


ACTS3 = DEFAULT_CFG["acts3"]


def _pack_inputs(inputs):
    um, vm = _build_masks()
    if ACTS3:
        # bank order (g,i,f,o), no cell pre-scaling (direct tanh on device)
        gates = [
            (inputs["U_c"], inputs["V_c"], inputs["b_c"]),
            (inputs["U_i"], inputs["V_i"], inputs["b_i"]),
            (inputs["U_f"], inputs["V_f"], inputs["b_f"]),
            (inputs["U_o"], inputs["V_o"], inputs["b_o"]),
        ]
    else:
        gates = [
            (inputs["U_i"], inputs["V_i"], inputs["b_i"]),
            (inputs["U_f"], inputs["V_f"], inputs["b_f"]),
            (inputs["U_o"], inputs["V_o"], inputs["b_o"]),
            (inputs["U_c"], inputs["V_c"], inputs["b_c"]),
        ]
    Up = [np.asarray(U, np.float32) * um for U, _, _ in gates]
    Vp = [np.asarray(V, np.float32) * vm for _, V, _ in gates]
    bs = [np.asarray(b, np.float32) for _, _, b in gates]
    # canonical (i,f,o,c) order for _host_tail, independent of device order
    canon = [
        (inputs["U_i"], inputs["V_i"], inputs["b_i"]),
        (inputs["U_f"], inputs["V_f"], inputs["b_f"]),
        (inputs["U_o"], inputs["V_o"], inputs["b_o"]),
        (inputs["U_c"], inputs["V_c"], inputs["b_c"]),
    ]
    Vp_host = [np.asarray(V, np.float32) * vm for _, V, _ in canon]
    bs_host = [np.asarray(b, np.float32) for _, _, b in canon]
    x_seq = np.asarray(inputs["x_seq"], np.float32)
    out_coef = np.asarray(inputs["out_coef"], np.float32)

    in_maps = []
    for core in range(N_CORES):
        feats = list(range(4 * core, 4 * core + 4))
        hs = slice(CORE_HID * core, CORE_HID * (core + 1))
        xf = np.ones((5, T * B), np.float32)
        # column index = t*B + b
        xf[0:4] = x_seq[:, :, feats].transpose(2, 1, 0).reshape(4, T * B)
        wu = np.zeros((4, 5, CORE_HID), np.float32)
        wv = np.zeros((4, CORE_HID, CORE_HID), np.float32)
        for g in range(4):
            wu[g, 0:4] = Up[g][feats, hs]
            if core == 0:
                # interaction rows 32,33 multiply x0,x1 -> fold into rows 0,1
                wu[g, 0] += Up[g][32, hs]
                wu[g, 1] += Up[g][33, hs]
            wu[g, 4] = bs[g][hs]
            wv[g] = Vp[g][hs, hs]
        if not ACTS3:
            # cell gate (idx 3) pre-scaled x2: tanh(y) = 2*sigmoid(2y) - 1
            wu[3] *= 2.0
            wv[3] *= 2.0
        in_maps.append(
            {
                "xf": xf.astype(np.float16),
                "wu": wu.astype(np.float16),
                "wv": wv.astype(np.float16),
                "oc": np.ascontiguousarray(out_coef[hs]).astype(np.float16),
            }
        )
    return in_maps, Vp_host, bs_host, out_coef


def _host_tail(inputs, partials, Vp, bs, out_coef):
    """Bias-only blocks 32,33 (batch-independent scalar) + static MLP +
    final sigmoid. All exact model math, done during unshard."""
    aux = slice(32 * HPF, HID)  # hid 1024:1088
    h = np.zeros(2 * HPF, np.float32)
    cst = np.zeros(2 * HPF, np.float32)
    Va = [V[aux, aux] for V in Vp]
    ba = [b[aux] for b in bs]

    def sig(x):
        return 1.0 / (1.0 + np.exp(-x))

    for _ in range(T):
        i_t = sig(ba[0] + h @ Va[0])
        f_t = sig(ba[1] + h @ Va[1])
        o_t = sig(ba[2] + h @ Va[2])
        g_t = np.tanh(ba[3] + h @ Va[3])
        cst = f_t * cst + i_t * g_t
        h = o_t * np.tanh(cst)
    s_aux = float(h @ out_coef[aux, 0])

    x_stat = np.asarray(inputs["x_stat"], np.float32)
    W1 = np.asarray(inputs["W1"], np.float32)
    b1 = np.asarray(inputs["b1"], np.float32)
    W2 = np.asarray(inputs["W2"], np.float32)
    b2 = np.asarray(inputs["b2"], np.float32)
    hid = np.maximum(x_stat[:, :, None] * W1[None] + b1[None], 0.0)
    mlp = sig(np.einsum("bfk,fk->bf", hid, W2) + b2)
    mlp_part = mlp @ out_coef[HID:, 0]

    z = partials.sum(axis=0) + s_aux + mlp_part + float(np.asarray(inputs["out_bias"])[0])
    return sig(z).astype(np.float32).reshape(B, 1)


def kernel(**inputs):
    from concourse.bass_utils import run_bass_kernel_spmd

    if "nc" not in _CACHE:
        _CACHE["nc"] = _build_program()
    nc = _CACHE["nc"]

    in_maps, Vp, bs, out_coef = _pack_inputs(inputs)
    res = run_bass_kernel_spmd(nc, in_maps, core_ids=list(range(N_CORES)))
    partials = np.stack([res.results[c]["partial"][0] for c in range(N_CORES)])
    return _host_tail(inputs, partials, Vp, bs, out_coef)


# revision 14
# speedup vs baseline: 1.1966x; 1.1966x over previous
"""Trainium2 Bass kernel for the masked block-diagonal LSTM net.

Model structure (hardcoded from the problem spec):
  - x_seq [512, 64, 32], recurrent state HID=1088 = 34 blocks x 32.
  - U projections are masked so hidden block j only sees input feature j
    (block 0 additionally sees features 0,1 again via the interaction rows);
    hidden blocks 32,33 receive NO input projection at all.
  - V recurrent matrices are masked block-diagonal -> the 34 blocks evolve
    completely independently through the scan.

Sharding: hidden-block parallel. Cores 0..7 each own 4 input-driven blocks
(128 hidden rows) x the full batch 512. Layout on device is h^T:
[hid on partitions, batch on free dim], so the recurrent matmul, the gate
activations and the state updates all run at full 128-partition width with
N=512 columns and no transposes anywhere.

Blocks 32,33 are bias-only (no x dependence): their state is identical for
every batch element, so their scalar contribution to the readout (and the
tiny 16-feature static MLP + final sigmoid) is folded into the host-side
unshard step.

v2 restructure (vs the first working version):
  - t=0 recurrent matmuls skipped entirely (h0 == 0).
  - input-projection matmuls for step t+1 are emitted during step t and
    paired per gate across the two batch chunks so the PE can reuse the
    loaded weights and stay busy while the elementwise chain runs.
  - scalar_tensor_tensor ops (no DVE perf modes, ~594ns measured) replaced
    with tensor_scalar immediate ops (4x mode) and a real Tanh activation:
      cell gate: g2 = 2*sigmoid(2y) - 1 == tanh(y)   (weights pre-scaled x2)
      h update:  h  = o * tanh(c)                     (direct Tanh act)
"""

import sys

sys.path.insert(0, "/opt/trn_rl_repo")

import numpy as np

B = 512
T = 64
INPUT_SZ = 32
HPF = 32
INTER = [(0, 1), (2, 3)]
NB = INPUT_SZ + len(INTER)  # 34
HID = NB * HPF  # 1088
IN_SZ = INPUT_SZ + 2 * len(INTER)  # 36
F_STAT = 16
N_CORES = 8
BLOCKS_PER_CORE = 4
CORE_HID = BLOCKS_PER_CORE * HPF  # 128
CHUNKS = 2  # batch-column chunks per step (pipelining granularity)
CB = B // CHUNKS

_CACHE = {}


def _build_masks():
    um = np.zeros((IN_SZ, HID), np.float32)
    for i in range(INPUT_SZ):
        um[i, i * HPF : (i + 1) * HPF] = 1.0
    for i in range(0, len(INTER), 2):
        um[i + INPUT_SZ, i * HPF : (i + 1) * HPF] = 1.0
        um[i + INPUT_SZ + 1, i * HPF : (i + 1) * HPF] = 1.0
    vm = np.kron(np.eye(NB, dtype=np.float32), np.ones((HPF, HPF), np.float32))
    return um, vm


DEFAULT_CFG = dict(
    acts3=False,   # 3-way act split with gate banks (g,i,f,o); else fused
                   # 4-bank sigmoid with x2-prescaled cell gate
    chunks=2,      # batch-column chunks per step
    t1_eng="v",    # engine for f*c: "v" (vector) or "g" (gpsimd)
    inp_pos="after_acts",  # where inp(t+1) MMs are emitted:
                           # "after_acts" | "before_acts" | "after_chain"
    merged=False,  # emit chain(ch) right after act(ch) in one loop
    wbufs=3,
    sbufs=2,
)


def _build_program(repeat=1, loop_n=0, cfg=None):
    # repeat>1 duplicates the whole computation serially (same I/O).
    # loop_n>0 instead wraps ONE copy in a hardware For_i loop executing
    # loop_n times: program size stays constant, so wall-clock deltas
    # between two loop_n values isolate true device execution time from
    # the per-call NEFF dispatch overhead (which scales with program size).
    import concourse.bass as bass
    import concourse.tile as tile
    from concourse import bacc, mybir
    from contextlib import nullcontext

    f32 = mybir.dt.float32
    f16 = mybir.dt.float16
    ACT = mybir.ActivationFunctionType
    ALU = mybir.AluOpType

    cfg = dict(DEFAULT_CFG, **(cfg or {}))
    CHUNKS = cfg["chunks"]
    CB = B // CHUNKS

    nc = bacc.Bacc("TRN2", target_bir_lowering=False, debug=False)

    xf_d = nc.dram_tensor("xf", [5, T * B], f16, kind="ExternalInput").ap()
    wu_d = nc.dram_tensor("wu", [4, 5, CORE_HID], f16, kind="ExternalInput").ap()
    wv_d = nc.dram_tensor("wv", [4, CORE_HID, CORE_HID], f16, kind="ExternalInput").ap()
    oc_d = nc.dram_tensor("oc", [CORE_HID, 1], f16, kind="ExternalInput").ap()
    part_d = nc.dram_tensor("partial", [1, B], f32, kind="ExternalOutput").ap()

    with tile.TileContext(nc) as tc:
        with (
            tc.tile_pool(name="const", bufs=1) as cpool,
            tc.tile_pool(name="state", bufs=cfg["sbufs"]) as spool,
            tc.tile_pool(name="work", bufs=cfg["wbufs"]) as wpool,
            tc.tile_pool(name="psum", bufs=2, space="PSUM") as ppool,
        ):
            # DMA order: tiny input-projection weights first, then the
            # t=0 slice of xf so the first matmuls can issue while the
            # recurrent weights and the rest of xf stream in behind them.
            wu = []
            wv = []
            xf = cpool.tile([5, T * B], f16, tag="xf")
            for g in range(4):
                wut = cpool.tile([5, CORE_HID], f16, tag=f"wu{g}")
                nc.sync.dma_start(wut[:], wu_d[g])
                wu.append(wut)
            nc.sync.dma_start(xf[:, 0:B], xf_d[:, 0:B])
            for g in range(4):
                wvt = cpool.tile([CORE_HID, CORE_HID], f16, tag=f"wv{g}")
                nc.sync.dma_start(wvt[:], wv_d[g])
                wv.append(wvt)
            for lo, hi in ((1, 4), (4, 16), (16, 64)):
                nc.sync.dma_start(xf[:, lo * B : hi * B], xf_d[:, lo * B : hi * B])
            oc = cpool.tile([CORE_HID, 1], f16, tag="oc")
            nc.sync.dma_start(oc[:], oc_d[:])

            def emit_inp(ps_tiles, t, stop):
                # input projections for step t, paired per gate across
                # chunks so the PE can reuse the loaded weights
                for g in range(4):
                    for ch in range(CHUNKS):
                        nc.tensor.matmul(
                            ps_tiles[ch][:, g],
                            wu[g][:],
                            xf[:, t * B + ch * CB : t * B + (ch + 1) * CB],
                            start=True,
                            stop=stop,
                        )

            loop_cm = (lambda: tc.For_i(0, loop_n, 1)) if loop_n else None
            for rep in range(repeat):
              with loop_cm() if loop_cm else nullcontext():
                cs_t = [None] * CHUNKS
                hs_t = [None] * CHUNKS

                ps_cur = [
                    ppool.tile([128, 4, CB], f32, tag=f"ps{ch}", name=f"ps{ch}")
                    for ch in range(CHUNKS)
                ]
                emit_inp(ps_cur, 0, stop=True)

                def emit_rec(ch, t):
                    if t > 0:
                        for g in range(4):
                            nc.tensor.matmul(
                                ps_cur[ch][:, g],
                                wv[g][:],
                                hs_t[ch][:],
                                start=False,
                                stop=True,
                            )

                def emit_act(ch):
                    if cfg["acts3"]:
                        # banks are (g,i,f,o): direct tanh on the cell bank
                        # first (its matmul lands first), then sigmoid(i,f),
                        # then sigmoid(o) -- separate tiles so the c-path
                        # never waits on the o-gate activation
                        tg = wpool.tile([CORE_HID, CB], f16, tag=f"tg{ch}", name=f"tg{ch}")
                        nc.scalar.activation(tg[:], ps_cur[ch][:, 0], ACT.Tanh)
                        sif = wpool.tile([CORE_HID, 2, CB], f16, tag=f"sif{ch}", name=f"sif{ch}")
                        nc.scalar.activation(sif[:], ps_cur[ch][:, 1:3], ACT.Sigmoid)
                        so = wpool.tile([CORE_HID, CB], f16, tag=f"so{ch}", name=f"so{ch}")
                        nc.scalar.activation(so[:], ps_cur[ch][:, 3], ACT.Sigmoid)
                        return (tg, sif, so)
                    # one fused sigmoid over all 4 gate banks; the cell
                    # gate's weights are pre-scaled x2 so bank 3 yields
                    # g' = sigmoid(2y) with tanh(y) = 2g' - 1
                    ifog = wpool.tile([CORE_HID, 4, CB], f16, tag=f"ifog{ch}", name=f"ifog{ch}")
                    nc.scalar.activation(ifog[:], ps_cur[ch][:], ACT.Sigmoid)
                    return ifog

                def emit_chain(ch, ifog, first=False):
                    if cfg["acts3"]:
                        tg, sif, so = ifog
                        t1 = wpool.tile([CORE_HID, CB], f16, tag=f"t1{ch}", name=f"t1{ch}")
                        nc.vector.tensor_mul(t1[:], sif[:, 1], cs_t[ch][:])  # f*c
                        t2 = wpool.tile([CORE_HID, CB], f16, tag=f"t2{ch}", name=f"t2{ch}")
                        nc.vector.tensor_mul(t2[:], sif[:, 0], tg[:])  # i*tanh(y)
                        c_new = spool.tile([CORE_HID, CB], f16, tag=f"c{ch}", name=f"c{ch}")
                        nc.vector.tensor_add(c_new[:], t1[:], t2[:])
                        sc = wpool.tile([CORE_HID, CB], f16, tag=f"sc{ch}", name=f"sc{ch}")
                        nc.scalar.activation(sc[:], c_new[:], ACT.Tanh)
                        h_new = spool.tile([CORE_HID, CB], f16, tag=f"h{ch}", name=f"h{ch}")
                        nc.vector.tensor_mul(h_new[:], so[:], sc[:])
                        hs_t[ch] = h_new
                        cs_t[ch] = c_new
                        return
                    i_, f_, o_, g_ = (ifog[:, k] for k in range(4))
                    # tanh(y) = 2*sigmoid(2y) - 1 (4x-mode tensor_scalar)
                    g2 = wpool.tile([CORE_HID, CB], f16, tag=f"g2{ch}", name=f"g2{ch}")
                    nc.vector.tensor_scalar(
                        g2[:], g_, 2.0, -1.0, ALU.mult, ALU.add
                    )
                    c_new = spool.tile([CORE_HID, CB], f16, tag=f"c{ch}", name=f"c{ch}")
                    if first:
                        # c0 == 0: c1 = i * tanh(y) directly
                        nc.vector.tensor_mul(c_new[:], i_, g2[:])
                    else:
                        t1 = wpool.tile([CORE_HID, CB], f16, tag=f"t1{ch}", name=f"t1{ch}")
                        t1_eng = nc.vector if cfg["t1_eng"] == "v" else nc.gpsimd
                        t1_eng.tensor_mul(t1[:], f_, cs_t[ch][:])  # f*c
                        t2 = wpool.tile([CORE_HID, CB], f16, tag=f"t2{ch}", name=f"t2{ch}")
                        nc.vector.tensor_mul(t2[:], i_, g2[:])  # i*tanh(y)
                        nc.vector.tensor_add(c_new[:], t1[:], t2[:])
                    sc = wpool.tile([CORE_HID, CB], f16, tag=f"sc{ch}", name=f"sc{ch}")
                    nc.scalar.activation(sc[:], c_new[:], ACT.Tanh)
                    h_new = spool.tile([CORE_HID, CB], f16, tag=f"h{ch}", name=f"h{ch}")
                    nc.vector.tensor_mul(h_new[:], o_, sc[:])
                    hs_t[ch] = h_new
                    cs_t[ch] = c_new

                for t in range(T):
                    def emit_inp_next():
                        if t + 1 < T:
                            pn = [
                                ppool.tile([128, 4, CB], f32, tag=f"ps{ch}", name=f"ps{ch}")
                                for ch in range(CHUNKS)
                            ]
                            emit_inp(pn, t + 1, stop=False)
                            return pn
                        return None

                    ps_next = None
                    if cfg["inp_pos"] == "before_acts":
                        for ch in range(CHUNKS):
                            emit_rec(ch, t)
                        ps_next = emit_inp_next()
                        ifogs = [emit_act(ch) for ch in range(CHUNKS)]
                        for ch in range(CHUNKS):
                            emit_chain(ch, ifogs[ch], first=(t == 0))
                    elif cfg["merged"]:
                        ifogs = [None] * CHUNKS
                        for ch in range(CHUNKS):
                            emit_rec(ch, t)
                            ifogs[ch] = emit_act(ch)
                            if ch == 0 and cfg["inp_pos"] == "after_acts":
                                ps_next = emit_inp_next()
                            emit_chain(ch, ifogs[ch], first=(t == 0))
                        if ps_next is None:
                            ps_next = emit_inp_next()
                    else:
                        ifogs = []
                        for ch in range(CHUNKS):
                            emit_rec(ch, t)
                            ifogs.append(emit_act(ch))
                        if cfg["inp_pos"] == "after_acts":
                            ps_next = emit_inp_next()
                        for ch in range(CHUNKS):
                            emit_chain(ch, ifogs[ch], first=(t == 0))
                        if cfg["inp_pos"] == "after_chain":
                            ps_next = emit_inp_next()

                    ps_cur = ps_next

                # readout partial: oc^T @ h  -> [1, B]
                outsb = wpool.tile([1, B], f32, tag="outsb")
                for ch in range(CHUNKS):
                    pr = ppool.tile([128, 4, CB], f32, tag=f"ps{ch}")
                    nc.tensor.matmul(
                        pr[0:1, 0], oc[:], hs_t[ch][:], start=True, stop=True
                    )
                    nc.vector.tensor_copy(outsb[:, ch * CB : (ch + 1) * CB], pr[0:1, 0])
                nc.sync.dma_start(part_d[:], outsb[:])

    nc.compile()
    return nc


ACTS3 = DEFAULT_CFG["acts3"]


def _pack_inputs(inputs):
    um, vm = _build_masks()
    if ACTS3:
        # bank order (g,i,f,o), no cell pre-scaling (direct tanh on device)
        gates = [
            (inputs["U_c"], inputs["V_c"], inputs["b_c"]),
            (inputs["U_i"], inputs["V_i"], inputs["b_i"]),
            (inputs["U_f"], inputs["V_f"], inputs["b_f"]),
            (inputs["U_o"], inputs["V_o"], inputs["b_o"]),
        ]
    else:
        gates = [
            (inputs["U_i"], inputs["V_i"], inputs["b_i"]),
            (inputs["U_f"], inputs["V_f"], inputs["b_f"]),
            (inputs["U_o"], inputs["V_o"], inputs["b_o"]),
            (inputs["U_c"], inputs["V_c"], inputs["b_c"]),
        ]
    Up = [np.asarray(U, np.float32) * um for U, _, _ in gates]
    Vp = [np.asarray(V, np.float32) * vm for _, V, _ in gates]
    bs = [np.asarray(b, np.float32) for _, _, b in gates]
    # canonical (i,f,o,c) order for _host_tail, independent of device order
    canon = [
        (inputs["U_i"], inputs["V_i"], inputs["b_i"]),
        (inputs["U_f"], inputs["V_f"], inputs["b_f"]),
        (inputs["U_o"], inputs["V_o"], inputs["b_o"]),
        (inputs["U_c"], inputs["V_c"], inputs["b_c"]),
    ]
    Vp_host = [np.asarray(V, np.float32) * vm for _, V, _ in canon]
    bs_host = [np.asarray(b, np.float32) for _, _, b in canon]
    x_seq = np.asarray(inputs["x_seq"], np.float32)
    out_coef = np.asarray(inputs["out_coef"], np.float32)

    in_maps = []
    for core in range(N_CORES):
        feats = list(range(4 * core, 4 * core + 4))
        hs = slice(CORE_HID * core, CORE_HID * (core + 1))
        xf = np.ones((5, T * B), np.float32)
        # column index = t*B + b
        xf[0:4] = x_seq[:, :, feats].transpose(2, 1, 0).reshape(4, T * B)
        wu = np.zeros((4, 5, CORE_HID), np.float32)
        wv = np.zeros((4, CORE_HID, CORE_HID), np.float32)
        for g in range(4):
            wu[g, 0:4] = Up[g][feats, hs]
            if core == 0:
                # interaction rows 32,33 multiply x0,x1 -> fold into rows 0,1
                wu[g, 0] += Up[g][32, hs]
                wu[g, 1] += Up[g][33, hs]
            wu[g, 4] = bs[g][hs]
            wv[g] = Vp[g][hs, hs]
        if not ACTS3:
            # cell gate (idx 3) pre-scaled x2: tanh(y) = 2*sigmoid(2y) - 1
            wu[3] *= 2.0
            wv[3] *= 2.0
        in_maps.append(
            {
                "xf": xf.astype(np.float16),
                "wu": wu.astype(np.float16),
                "wv": wv.astype(np.float16),
                "oc": np.ascontiguousarray(out_coef[hs]).astype(np.float16),
            }
        )
    return in_maps, Vp_host, bs_host, out_coef


def _host_tail(inputs, partials, Vp, bs, out_coef):
    """Bias-only blocks 32,33 (batch-independent scalar) + static MLP +
    final sigmoid. All exact model math, done during unshard."""
    aux = slice(32 * HPF, HID)  # hid 1024:1088
    h = np.zeros(2 * HPF, np.float32)
    cst = np.zeros(2 * HPF, np.float32)
    Va = [V[aux, aux] for V in Vp]
    ba = [b[aux] for b in bs]

    def sig(x):
        return 1.0 / (1.0 + np.exp(-x))

    for _ in range(T):
        i_t = sig(ba[0] + h @ Va[0])
        f_t = sig(ba[1] + h @ Va[1])
        o_t = sig(ba[2] + h @ Va[2])
        g_t = np.tanh(ba[3] + h @ Va[3])
        cst = f_t * cst + i_t * g_t
        h = o_t * np.tanh(cst)
    s_aux = float(h @ out_coef[aux, 0])

    x_stat = np.asarray(inputs["x_stat"], np.float32)
    W1 = np.asarray(inputs["W1"], np.float32)
    b1 = np.asarray(inputs["b1"], np.float32)
    W2 = np.asarray(inputs["W2"], np.float32)
    b2 = np.asarray(inputs["b2"], np.float32)
    hid = np.maximum(x_stat[:, :, None] * W1[None] + b1[None], 0.0)
    mlp = sig(np.einsum("bfk,fk->bf", hid, W2) + b2)
    mlp_part = mlp @ out_coef[HID:, 0]

    z = partials.sum(axis=0) + s_aux + mlp_part + float(np.asarray(inputs["out_bias"])[0])
    return sig(z).astype(np.float32).reshape(B, 1)


def kernel(**inputs):
    from concourse.bass_utils import run_bass_kernel_spmd

    if "nc" not in _CACHE:
        _CACHE["nc"] = _build_program()
    nc = _CACHE["nc"]

    in_maps, Vp, bs, out_coef = _pack_inputs(inputs)
    res = run_bass_kernel_spmd(nc, in_maps, core_ids=list(range(N_CORES)))
    partials = np.stack([res.results[c]["partial"][0] for c in range(N_CORES)])
    return _host_tail(inputs, partials, Vp, bs, out_coef)


# revision 19
# speedup vs baseline: 1.1979x; 1.0011x over previous
"""Trainium2 Bass kernel for the masked block-diagonal LSTM net.

Model structure (hardcoded from the problem spec):
  - x_seq [512, 64, 32], recurrent state HID=1088 = 34 blocks x 32.
  - U projections are masked so hidden block j only sees input feature j
    (block 0 additionally sees features 0,1 again via the interaction rows);
    hidden blocks 32,33 receive NO input projection at all.
  - V recurrent matrices are masked block-diagonal -> the 34 blocks evolve
    completely independently through the scan.

Sharding: hidden-block parallel. Cores 0..7 each own 4 input-driven blocks
(128 hidden rows) x the full batch 512. Layout on device is h^T:
[hid on partitions, batch on free dim], so the recurrent matmul, the gate
activations and the state updates all run at full 128-partition width with
N=512 columns and no transposes anywhere.

Blocks 32,33 are bias-only (no x dependence): their state is identical for
every batch element, so their scalar contribution to the readout (and the
tiny 16-feature static MLP + final sigmoid) is folded into the host-side
unshard step.

Optimization history (HW-measured on trn2, 8 cores, worst-core time):
  454-515us  baseline: per-step 16 matmuls + fused sigmoid + 2x
             scalar_tensor_tensor (STT) chain, Pool f*c.
  300us      v2: STT ops (no DVE perf modes, 594ns each) replaced by
             tensor_scalar immediates (4x DVE mode, 228ns) + real Tanh
             act for h; t=0 recurrent matmuls skipped (h0=0); inp
             matmuls for t+1 emitted during step t.
  293us      DMA preamble reordered (tiny wu weights + t=0 xf slice
             first, rest streams behind) so the first matmul issues at
             ~9us instead of ~19us; t=0 chain shortcut (c1 = i*g).

The steady state is latency-bound on the per-stream critical path
(~4.25us/step x 64 steps): rec-matmul burst (1.05us) -> sigmoid over
the 4 gate banks (1.11us) -> DVE chain g2/t1/t2/add (1.08us) ->
Tanh(c) (0.51us) -> h=o*sc (0.28us) + semaphore hops. Two batch-column
streams (CB=256) hide each other's engine time; PSUM (8 banks) holds
2 streams x 2 pipelined steps exactly.

Measured dead ends (all slower on HW): splitting the 4-bank sigmoid
into 2-3 activation insts (~300ns fixed overhead per Act inst),
f*c on GpSimd (Pool TT ~700ns sits on the c-path), chunks=4 (LdW and
per-inst overheads), deeper tile pools (wbufs=4/sbufs=3 -> 351us).
"""

import sys

sys.path.insert(0, "/opt/trn_rl_repo")

import numpy as np

B = 512
T = 64
INPUT_SZ = 32
HPF = 32
INTER = [(0, 1), (2, 3)]
NB = INPUT_SZ + len(INTER)  # 34
HID = NB * HPF  # 1088
IN_SZ = INPUT_SZ + 2 * len(INTER)  # 36
F_STAT = 16
N_CORES = 8
BLOCKS_PER_CORE = 4
CORE_HID = BLOCKS_PER_CORE * HPF  # 128
CHUNKS = 2  # batch-column chunks per step (pipelining granularity)
CB = B // CHUNKS

_CACHE = {}


def _build_masks():
    um = np.zeros((IN_SZ, HID), np.float32)
    for i in range(INPUT_SZ):
        um[i, i * HPF : (i + 1) * HPF] = 1.0
    for i in range(0, len(INTER), 2):
        um[i + INPUT_SZ, i * HPF : (i + 1) * HPF] = 1.0
        um[i + INPUT_SZ + 1, i * HPF : (i + 1) * HPF] = 1.0
    vm = np.kron(np.eye(NB, dtype=np.float32), np.ones((HPF, HPF), np.float32))
    return um, vm


DEFAULT_CFG = dict(
    acts3=False,   # 3-way act split with gate banks (g,i,f,o); else fused
                   # 4-bank sigmoid with x2-prescaled cell gate
    chunks=2,      # batch-column chunks per step
    t1_eng="v",    # engine for f*c: "v" (vector) or "g" (gpsimd)
    inp_pos="after_acts",  # where inp(t+1) MMs are emitted:
                           # "after_acts" | "before_acts" | "after_chain"
    merged=False,  # emit chain(ch) right after act(ch) in one loop
    wbufs=3,
    sbufs=2,
    tail_prio=False,  # schedule add/tanh/h ahead of the other chunk's
                      # chain heads when both are ready (SRTF-style)
    tail2=True,    # split tanh(c)/h and the rec matmuls into column
                   # halves so rec half-a starts while half-b finishes
)


def _build_program(repeat=1, loop_n=0, cfg=None):
    # repeat>1 duplicates the whole computation serially (same I/O).
    # loop_n>0 instead wraps ONE copy in a hardware For_i loop executing
    # loop_n times: program size stays constant, so wall-clock deltas
    # between two loop_n values isolate true device execution time from
    # the per-call NEFF dispatch overhead (which scales with program size).
    import concourse.bass as bass
    import concourse.tile as tile
    from concourse import bacc, mybir
    from contextlib import nullcontext

    f32 = mybir.dt.float32
    f16 = mybir.dt.float16
    ACT = mybir.ActivationFunctionType
    ALU = mybir.AluOpType

    cfg = dict(DEFAULT_CFG, **(cfg or {}))
    CHUNKS = cfg["chunks"]
    CB = B // CHUNKS

    nc = bacc.Bacc("TRN2", target_bir_lowering=False, debug=False)

    xf_d = nc.dram_tensor("xf", [5, T * B], f16, kind="ExternalInput").ap()
    wu_d = nc.dram_tensor("wu", [4, 5, CORE_HID], f16, kind="ExternalInput").ap()
    wv_d = nc.dram_tensor("wv", [4, CORE_HID, CORE_HID], f16, kind="ExternalInput").ap()
    oc_d = nc.dram_tensor("oc", [CORE_HID, 1], f16, kind="ExternalInput").ap()
    part_d = nc.dram_tensor("partial", [1, B], f32, kind="ExternalOutput").ap()

    with tile.TileContext(nc) as tc:
        with (
            tc.tile_pool(name="const", bufs=1) as cpool,
            tc.tile_pool(name="state", bufs=cfg["sbufs"]) as spool,
            tc.tile_pool(name="work", bufs=cfg["wbufs"]) as wpool,
            tc.tile_pool(name="psum", bufs=2, space="PSUM") as ppool,
        ):
            # DMA order: tiny input-projection weights first, then the
            # t=0 slice of xf so the first matmuls can issue while the
            # recurrent weights and the rest of xf stream in behind them.
            wu = []
            wv = []
            xf = cpool.tile([5, T * B], f16, tag="xf")
            for g in range(4):
                wut = cpool.tile([5, CORE_HID], f16, tag=f"wu{g}")
                nc.sync.dma_start(wut[:], wu_d[g])
                wu.append(wut)
            nc.sync.dma_start(xf[:, 0:B], xf_d[:, 0:B])
            for g in range(4):
                wvt = cpool.tile([CORE_HID, CORE_HID], f16, tag=f"wv{g}")
                nc.sync.dma_start(wvt[:], wv_d[g])
                wv.append(wvt)
            for lo, hi in ((1, 4), (4, 16), (16, 64)):
                nc.sync.dma_start(xf[:, lo * B : hi * B], xf_d[:, lo * B : hi * B])
            oc = cpool.tile([CORE_HID, 1], f16, tag="oc")
            nc.sync.dma_start(oc[:], oc_d[:])

            def emit_inp(ps_tiles, t, stop):
                # input projections for step t, paired per gate across
                # chunks so the PE can reuse the loaded weights
                for g in range(4):
                    for ch in range(CHUNKS):
                        nc.tensor.matmul(
                            ps_tiles[ch][:, g],
                            wu[g][:],
                            xf[:, t * B + ch * CB : t * B + (ch + 1) * CB],
                            start=True,
                            stop=stop,
                        )

            loop_cm = (lambda: tc.For_i(0, loop_n, 1)) if loop_n else None
            for rep in range(repeat):
              with loop_cm() if loop_cm else nullcontext():
                cs_t = [None] * CHUNKS
                hs_t = [None] * CHUNKS

                ps_cur = [
                    ppool.tile([128, 4, CB], f32, tag=f"ps{ch}", name=f"ps{ch}")
                    for ch in range(CHUNKS)
                ]
                emit_inp(ps_cur, 0, stop=True)

                def emit_rec(ch, t):
                    if t > 0:
                        if cfg["tail2"]:
                            HB = CB // 2
                            for j in range(2):
                                hj = hs_t[ch][j]
                                for g in range(4):
                                    nc.tensor.matmul(
                                        ps_cur[ch][:, g, j * HB : (j + 1) * HB],
                                        wv[g][:],
                                        hj[:],
                                        start=False,
                                        stop=True,
                                    )
                        else:
                            for g in range(4):
                                nc.tensor.matmul(
                                    ps_cur[ch][:, g],
                                    wv[g][:],
                                    hs_t[ch][:],
                                    start=False,
                                    stop=True,
                                )

                def emit_act(ch):
                    if cfg["acts3"]:
                        # banks are (g,i,f,o): direct tanh on the cell bank
                        # first (its matmul lands first), then sigmoid(i,f),
                        # then sigmoid(o) -- separate tiles so the c-path
                        # never waits on the o-gate activation
                        tg = wpool.tile([CORE_HID, CB], f16, tag=f"tg{ch}", name=f"tg{ch}")
                        nc.scalar.activation(tg[:], ps_cur[ch][:, 0], ACT.Tanh)
                        sif = wpool.tile([CORE_HID, 2, CB], f16, tag=f"sif{ch}", name=f"sif{ch}")
                        nc.scalar.activation(sif[:], ps_cur[ch][:, 1:3], ACT.Sigmoid)
                        so = wpool.tile([CORE_HID, CB], f16, tag=f"so{ch}", name=f"so{ch}")
                        nc.scalar.activation(so[:], ps_cur[ch][:, 3], ACT.Sigmoid)
                        return (tg, sif, so)
                    # one fused sigmoid over all 4 gate banks; the cell
                    # gate's weights are pre-scaled x2 so bank 3 yields
                    # g' = sigmoid(2y) with tanh(y) = 2g' - 1
                    ifog = wpool.tile([CORE_HID, 4, CB], f16, tag=f"ifog{ch}", name=f"ifog{ch}")
                    nc.scalar.activation(ifog[:], ps_cur[ch][:], ACT.Sigmoid)
                    return ifog

                def emit_chain(ch, ifog, first=False):
                    if cfg["acts3"]:
                        tg, sif, so = ifog
                        t1 = wpool.tile([CORE_HID, CB], f16, tag=f"t1{ch}", name=f"t1{ch}")
                        nc.vector.tensor_mul(t1[:], sif[:, 1], cs_t[ch][:])  # f*c
                        t2 = wpool.tile([CORE_HID, CB], f16, tag=f"t2{ch}", name=f"t2{ch}")
                        nc.vector.tensor_mul(t2[:], sif[:, 0], tg[:])  # i*tanh(y)
                        c_new = spool.tile([CORE_HID, CB], f16, tag=f"c{ch}", name=f"c{ch}")
                        nc.vector.tensor_add(c_new[:], t1[:], t2[:])
                        sc = wpool.tile([CORE_HID, CB], f16, tag=f"sc{ch}", name=f"sc{ch}")
                        nc.scalar.activation(sc[:], c_new[:], ACT.Tanh)
                        h_new = spool.tile([CORE_HID, CB], f16, tag=f"h{ch}", name=f"h{ch}")
                        nc.vector.tensor_mul(h_new[:], so[:], sc[:])
                        hs_t[ch] = h_new
                        cs_t[ch] = c_new
                        return
                    i_, f_, o_, g_ = (ifog[:, k] for k in range(4))
                    # tanh(y) = 2*sigmoid(2y) - 1 (4x-mode tensor_scalar)
                    g2 = wpool.tile([CORE_HID, CB], f16, tag=f"g2{ch}", name=f"g2{ch}")
                    nc.vector.tensor_scalar(
                        g2[:], g_, 2.0, -1.0, ALU.mult, ALU.add
                    )
                    from contextlib import nullcontext as _nl
                    prio = (lambda: tc.high_priority()) if cfg["tail_prio"] else _nl
                    c_new = spool.tile([CORE_HID, CB], f16, tag=f"c{ch}", name=f"c{ch}")
                    if first:
                        # c0 == 0: c1 = i * tanh(y) directly
                        nc.vector.tensor_mul(c_new[:], i_, g2[:])
                    else:
                        t1 = wpool.tile([CORE_HID, CB], f16, tag=f"t1{ch}", name=f"t1{ch}")
                        t1_eng = nc.vector if cfg["t1_eng"] == "v" else nc.gpsimd
                        t1_eng.tensor_mul(t1[:], f_, cs_t[ch][:])  # f*c
                        t2 = wpool.tile([CORE_HID, CB], f16, tag=f"t2{ch}", name=f"t2{ch}")
                        nc.vector.tensor_mul(t2[:], i_, g2[:])  # i*tanh(y)
                        with prio():
                            nc.vector.tensor_add(c_new[:], t1[:], t2[:])
                    if cfg["tail2"]:
                        HB = CB // 2
                        hs = []
                        for j in range(2):
                            scj = wpool.tile([CORE_HID, HB], f16, tag=f"sc{ch}_{j}", name=f"sc{ch}_{j}")
                            nc.scalar.activation(scj[:], c_new[:, j * HB : (j + 1) * HB], ACT.Tanh)
                            hj = spool.tile([CORE_HID, HB], f16, tag=f"h{ch}_{j}", name=f"h{ch}_{j}")
                            nc.vector.tensor_mul(hj[:], ifog[:, 2, j * HB : (j + 1) * HB], scj[:])
                            hs.append(hj)
                        hs_t[ch] = tuple(hs)
                    else:
                        sc = wpool.tile([CORE_HID, CB], f16, tag=f"sc{ch}", name=f"sc{ch}")
                        h_new = spool.tile([CORE_HID, CB], f16, tag=f"h{ch}", name=f"h{ch}")
                        with prio():
                            nc.scalar.activation(sc[:], c_new[:], ACT.Tanh)
                            nc.vector.tensor_mul(h_new[:], o_, sc[:])
                        hs_t[ch] = h_new
                    cs_t[ch] = c_new

                for t in range(T):
                    def emit_inp_next():
                        if t + 1 < T:
                            pn = [
                                ppool.tile([128, 4, CB], f32, tag=f"ps{ch}", name=f"ps{ch}")
                                for ch in range(CHUNKS)
                            ]
                            emit_inp(pn, t + 1, stop=False)
                            return pn
                        return None

                    ps_next = None
                    if cfg["inp_pos"] == "before_acts":
                        for ch in range(CHUNKS):
                            emit_rec(ch, t)
                        ps_next = emit_inp_next()
                        ifogs = [emit_act(ch) for ch in range(CHUNKS)]
                        for ch in range(CHUNKS):
                            emit_chain(ch, ifogs[ch], first=(t == 0))
                    elif cfg["merged"]:
                        ifogs = [None] * CHUNKS
                        for ch in range(CHUNKS):
                            emit_rec(ch, t)
                            ifogs[ch] = emit_act(ch)
                            if ch == 0 and cfg["inp_pos"] == "after_acts":
                                ps_next = emit_inp_next()
                            emit_chain(ch, ifogs[ch], first=(t == 0))
                        if ps_next is None:
                            ps_next = emit_inp_next()
                    else:
                        ifogs = []
                        for ch in range(CHUNKS):
                            emit_rec(ch, t)
                            ifogs.append(emit_act(ch))
                        if cfg["inp_pos"] == "after_acts":
                            ps_next = emit_inp_next()
                        for ch in range(CHUNKS):
                            emit_chain(ch, ifogs[ch], first=(t == 0))
                        if cfg["inp_pos"] == "after_chain":
                            ps_next = emit_inp_next()

                    ps_cur = ps_next

                # readout partial: oc^T @ h  -> [1, B]
                outsb = wpool.tile([1, B], f32, tag="outsb")
                for ch in range(CHUNKS):
                    pr = ppool.tile([128, 4, CB], f32, tag=f"ps{ch}")
                    if cfg["tail2"]:
                        HB = CB // 2
                        for j in range(2):
                            nc.tensor.matmul(
                                pr[0:1, 0, j * HB : (j + 1) * HB],
                                oc[:],
                                hs_t[ch][j][:],
                                start=True,
                                stop=True,
                            )
                    else:
                        nc.tensor.matmul(
                            pr[0:1, 0], oc[:], hs_t[ch][:], start=True, stop=True
                        )
                    nc.vector.tensor_copy(outsb[:, ch * CB : (ch + 1) * CB], pr[0:1, 0])
                nc.sync.dma_start(part_d[:], outsb[:])

    nc.compile()
    return nc


ACTS3 = DEFAULT_CFG["acts3"]


def _pack_inputs(inputs):
    um, vm = _build_masks()
    if ACTS3:
        # bank order (g,i,f,o), no cell pre-scaling (direct tanh on device)
        gates = [
            (inputs["U_c"], inputs["V_c"], inputs["b_c"]),
            (inputs["U_i"], inputs["V_i"], inputs["b_i"]),
            (inputs["U_f"], inputs["V_f"], inputs["b_f"]),
            (inputs["U_o"], inputs["V_o"], inputs["b_o"]),
        ]
    else:
        gates = [
            (inputs["U_i"], inputs["V_i"], inputs["b_i"]),
            (inputs["U_f"], inputs["V_f"], inputs["b_f"]),
            (inputs["U_o"], inputs["V_o"], inputs["b_o"]),
            (inputs["U_c"], inputs["V_c"], inputs["b_c"]),
        ]
    Up = [np.asarray(U, np.float32) * um for U, _, _ in gates]
    Vp = [np.asarray(V, np.float32) * vm for _, V, _ in gates]
    bs = [np.asarray(b, np.float32) for _, _, b in gates]
    # canonical (i,f,o,c) order for _host_tail, independent of device order
    canon = [
        (inputs["U_i"], inputs["V_i"], inputs["b_i"]),
        (inputs["U_f"], inputs["V_f"], inputs["b_f"]),
        (inputs["U_o"], inputs["V_o"], inputs["b_o"]),
        (inputs["U_c"], inputs["V_c"], inputs["b_c"]),
    ]
    Vp_host = [np.asarray(V, np.float32) * vm for _, V, _ in canon]
    bs_host = [np.asarray(b, np.float32) for _, _, b in canon]
    x_seq = np.asarray(inputs["x_seq"], np.float32)
    out_coef = np.asarray(inputs["out_coef"], np.float32)

    in_maps = []
    for core in range(N_CORES):
        feats = list(range(4 * core, 4 * core + 4))
        hs = slice(CORE_HID * core, CORE_HID * (core + 1))
        xf = np.ones((5, T * B), np.float32)
        # column index = t*B + b
        xf[0:4] = x_seq[:, :, feats].transpose(2, 1, 0).reshape(4, T * B)
        wu = np.zeros((4, 5, CORE_HID), np.float32)
        wv = np.zeros((4, CORE_HID, CORE_HID), np.float32)
        for g in range(4):
            wu[g, 0:4] = Up[g][feats, hs]
            if core == 0:
                # interaction rows 32,33 multiply x0,x1 -> fold into rows 0,1
                wu[g, 0] += Up[g][32, hs]
                wu[g, 1] += Up[g][33, hs]
            wu[g, 4] = bs[g][hs]
            wv[g] = Vp[g][hs, hs]
        if not ACTS3:
            # cell gate (idx 3) pre-scaled x2: tanh(y) = 2*sigmoid(2y) - 1
            wu[3] *= 2.0
            wv[3] *= 2.0
        in_maps.append(
            {
                "xf": xf.astype(np.float16),
                "wu": wu.astype(np.float16),
                "wv": wv.astype(np.float16),
                "oc": np.ascontiguousarray(out_coef[hs]).astype(np.float16),
            }
        )
    return in_maps, Vp_host, bs_host, out_coef


def _host_tail(inputs, partials, Vp, bs, out_coef):
    """Bias-only blocks 32,33 (batch-independent scalar) + static MLP +
    final sigmoid. All exact model math, done during unshard."""
    aux = slice(32 * HPF, HID)  # hid 1024:1088
    h = np.zeros(2 * HPF, np.float32)
    cst = np.zeros(2 * HPF, np.float32)
    Va = [V[aux, aux] for V in Vp]
    ba = [b[aux] for b in bs]

    def sig(x):
        return 1.0 / (1.0 + np.exp(-x))

    for _ in range(T):
        i_t = sig(ba[0] + h @ Va[0])
        f_t = sig(ba[1] + h @ Va[1])
        o_t = sig(ba[2] + h @ Va[2])
        g_t = np.tanh(ba[3] + h @ Va[3])
        cst = f_t * cst + i_t * g_t
        h = o_t * np.tanh(cst)
    s_aux = float(h @ out_coef[aux, 0])

    x_stat = np.asarray(inputs["x_stat"], np.float32)
    W1 = np.asarray(inputs["W1"], np.float32)
    b1 = np.asarray(inputs["b1"], np.float32)
    W2 = np.asarray(inputs["W2"], np.float32)
    b2 = np.asarray(inputs["b2"], np.float32)
    hid = np.maximum(x_stat[:, :, None] * W1[None] + b1[None], 0.0)
    mlp = sig(np.einsum("bfk,fk->bf", hid, W2) + b2)
    mlp_part = mlp @ out_coef[HID:, 0]

    z = partials.sum(axis=0) + s_aux + mlp_part + float(np.asarray(inputs["out_bias"])[0])
    return sig(z).astype(np.float32).reshape(B, 1)


def kernel(**inputs):
    from concourse.bass_utils import run_bass_kernel_spmd

    if "nc" not in _CACHE:
        _CACHE["nc"] = _build_program()
    nc = _CACHE["nc"]

    in_maps, Vp, bs, out_coef = _pack_inputs(inputs)
    res = run_bass_kernel_spmd(nc, in_maps, core_ids=list(range(N_CORES)))
    partials = np.stack([res.results[c]["partial"][0] for c in range(N_CORES)])
    return _host_tail(inputs, partials, Vp, bs, out_coef)
